# revision 8
# baseline (speedup 1.0000x reference)
"""CLIP encoder layer (LN -> causal MHA -> residual -> LN -> quickGELU MLP -> residual)
on 8 Trainium2 NeuronCores, SPMD via bass/Tile. v2.

Sharding: 8 shards = 4 batches x 2 query-groups. Core c handles batch c//2 with
parity s = c%2, owning 1024 query tokens in 4 phase-groups of 256. Each core
recomputes LN1 + K/V for the full sequence of its batch (no collectives).

4-phase causal schedule: phase p attends its 256 own queries against the first
EXT[p] = (4,8,12,16) key tiles of 128. A per-parity permutation of 256-token
blocks (pairs swapped) makes the local key order prefix-consistent for both
parities while own queries sit at uniform local offsets 256+512p. Only the last
4 key tiles of each phase are masked (diagonal + parity waste), via data.

Everything on the matmul path is bf16 (tolerance 2e-2 allows it): LN outputs
bf16, transposed token-major -> feature-major by the DMA XBAR transpose (14ns/
tile, zero engine cost) instead of PE transposes. K stays resident in SBUF (no
DRAM spill), y (residual stream) stays resident in SBUF. Softmax uses the
augmented-V ones-row trick for sums; the reciprocal is broadcast across
partitions by GPSIMD partition_broadcast (no PE/DVE broadcast work). Weights
are pre-permuted on the host; LN affines, q-scale and quickGELU's 1.702 are
folded into weights/biases.
"""

import sys

sys.path.insert(0, "/opt/trn_rl_repo")

import numpy as np
import ml_dtypes

import concourse.bass as bass
import concourse.mybir as mybir
import concourse.tile as tile
from concourse import bacc
from concourse.bass_utils import run_bass_kernel_spmd

B, S, D = 4, 2048, 1024
H, HD = 16, 64
NCORES = 8
EPS = 1e-5
OWN = 1024
EXT = [4, 8, 12, 16]
F32 = mybir.dt.float32
BF16 = mybir.dt.bfloat16
FP8 = mybir.dt.float8e4
ALU = mybir.AluOpType
AF = mybir.ActivationFunctionType

_CACHE = {}


def _bcast(ap1d, p=128):
    """[N] DRAM vector viewed as [p, N] with partition-step 0 (broadcast load)."""
    return bass.AP(tensor=ap1d.tensor, offset=ap1d.offset, ap=[[0, p]] + list(ap1d.ap))


def _build_program():
    nc = bacc.Bacc("TRN2", target_bir_lowering=False, debug=False,
                   num_devices=NCORES)

    t = {}
    t["xb"] = nc.dram_tensor("xb", [S, D], F32, kind="ExternalInput").ap()
    t["xh"] = nc.dram_tensor("xh", [S, D], BF16, kind="ExternalInput").ap()
    t["wq8"] = nc.dram_tensor("wq8", [8, 128, D], BF16, kind="ExternalInput").ap()
    t["wk8"] = nc.dram_tensor("wk8", [8, 128, D], BF16, kind="ExternalInput").ap()
    t["wvd"] = nc.dram_tensor("wvd", [128, 2, 8, 512], BF16, kind="ExternalInput").ap()
    t["wod"] = nc.dram_tensor("wod", [128, 8 * D], BF16, kind="ExternalInput").ap()
    t["w1t"] = nc.dram_tensor("w1t", [32, 128, D], BF16, kind="ExternalInput").ap()
    t["w2d"] = nc.dram_tensor("w2d", [128, 32 * D], BF16, kind="ExternalInput").ap()
    t["mskd"] = nc.dram_tensor("mskd", [128, 16 * 256], BF16, kind="ExternalInput").ap()
    t["bqd"] = nc.dram_tensor("bqd", [128, 8], F32, kind="ExternalInput").ap()
    t["bkd"] = nc.dram_tensor("bkd", [128, 8], F32, kind="ExternalInput").ap()
    t["b1d"] = nc.dram_tensor("b1d", [128, 32], F32, kind="ExternalInput").ap()
    for b_ in ("bo", "b2"):
        t[b_] = nc.dram_tensor(b_, [D], F32, kind="ExternalInput").ap()
    t["yo"] = nc.dram_tensor("yo", [OWN, D], F32, kind="ExternalOutput").ap()

    with tile.TileContext(nc) as tc:
        _body(nc, tc, t)
    nc.compile()
    return nc


def _ln_normalize(nc, stat, eps_t, x_t, out_t, tag, on_dve=False):
    """LayerNorm (affine folded into downstream weights): out = (x-mu)*rstd.
    Stats on DVE; the big normalize pass on ACT, or on DVE (tensor_scalar)
    when ACT is the busier engine (attention-phase LN2)."""
    st = stat.tile([128, 2, 6], F32, tag=f"{tag}st")
    for g in range(2):
        nc.vector.bn_stats(out=st[:, g, :], in_=x_t[:, g * 512:(g + 1) * 512])
    mv = stat.tile([128, 2], F32, tag=f"{tag}mv")
    nc.vector.bn_aggr(out=mv, in_=st)
    rstd = stat.tile([128, 1], F32, tag=f"{tag}rs")
    nc.scalar.activation(out=rstd, in_=mv[:, 1:2], func=AF.Sqrt, bias=eps_t, scale=1.0)
    nc.vector.reciprocal(out=rstd, in_=rstd)
    nmu = stat.tile([128, 1], F32, tag=f"{tag}nm")
    nc.vector.tensor_scalar(out=nmu, in0=mv[:, 0:1], scalar1=rstd, scalar2=-1.0,
                            op0=ALU.mult, op1=ALU.mult)
    if on_dve:
        nc.vector.tensor_scalar(out=out_t, in0=x_t, scalar1=rstd, scalar2=nmu,
                                op0=ALU.mult, op1=ALU.add)
    else:
        nc.scalar.activation(out=out_t, in_=x_t, func=AF.Identity, bias=nmu,
                             scale=rstd)


def _body(nc, tc, t):
    with tc.tile_pool(name="const", bufs=1) as const:
        eps_t = const.tile([128, 1], F32)
        nc.vector.memset(eps_t, EPS)
        bq_t = const.tile([128, 8], F32)
        bk_t = const.tile([128, 8], F32)
        b1s_t = const.tile([128, 32], F32)

        with tc.tile_pool(name="ysbp", bufs=1) as ysbp, \
             tc.tile_pool(name="h2p", bufs=1) as h2p, \
             tc.tile_pool(name="ln2s", bufs=6) as stat2, \
             tc.tile_pool(name="ln2w", bufs=3) as wrk2:
            y_sb = ysbp.tile([128, 8, D], F32)
            h2sb = [h2p.tile([128, 8, 256], BF16, tag=f"h2sb{c}",
                             name=f"h2sb{c}") for c in range(4)]

            with tc.tile_pool(name="vaugp", bufs=1) as vaugp, \
                 tc.tile_pool(name="qfmp", bufs=1) as qfmp, \
                 tc.tile_pool(name="ksbp", bufs=1) as ksbp:
                vaug = vaugp.tile([128, 16, H, 65], BF16)
                q_fm = qfmp.tile([128, 8, OWN], BF16)
                ksb = ksbp.tile([128, 8, S], BF16)

                nc.vector.memset(vaug[:, :, :, 64:65], 1.0)

                # ===== P0+P1 fused: LN1 / V / K / Q =====
                with tc.tile_pool(name="p01h1", bufs=1) as h1p, \
                     tc.tile_pool(name="p01x", bufs=6) as xpool, \
                     tc.tile_pool(name="p01s", bufs=6) as stat, \
                     tc.tile_pool(name="p01wv", bufs=2) as wvp, \
                     tc.tile_pool(name="p01wst", bufs=5) as wstr, \
                     tc.tile_pool(name="p01vps", bufs=2, space="PSUM") as vps, \
                     tc.tile_pool(name="p01kps", bufs=2, space="PSUM") as kps, \
                     tc.tile_pool(name="p01qps", bufs=2, space="PSUM") as qps:
                    h1_fm = h1p.tile([128, 8, S], BF16)

                    def ln1_tile(tt, on_dve):
                        x_t = xpool.tile([128, D], BF16, tag="x", name="x_t")
                        xq = nc.sync if tt < 2 else nc.gpsimd
                        xq.dma_start(
                            out=x_t, in_=t["xh"][tt * 128:(tt + 1) * 128, :])
                        _ln_normalize(nc, stat, eps_t, x_t, x_t, "a",
                                      on_dve=on_dve)
                        nc.sync.dma_start_transpose(
                            out=h1_fm[:, :, tt * 128:(tt + 1) * 128], in_=x_t)

                    def load_wv(fb):
                        wvh = wvp.tile([128, 8, 512], BF16, tag="wv", name="wvh")
                        nc.gpsimd.dma_start(out=wvh, in_=t["wvd"][:, fb, :, :])
                        return wvh

                    def v_tile(wvh, fb, tt, on_act):
                        ps = vps.tile([128, 512], F32, tag="ps", name="vps")
                        for kt in range(8):
                            nc.tensor.matmul(
                                ps, h1_fm[:, kt, tt * 128:(tt + 1) * 128],
                                wvh[:, kt, :],
                                start=(kt == 0), stop=(kt == 7))
                        dst = vaug[:, tt, fb * 8:(fb + 1) * 8, 0:64]
                        if on_act:
                            nc.scalar.copy(
                                out=dst, in_=ps.rearrange("p (h f) -> p h f", h=8))
                        else:
                            nc.vector.tensor_copy(
                                out=dst, in_=ps.rearrange("p (h f) -> p h f", h=8))

                    def k_pass(qbs, on_act):
                        for m in range(8):
                            wkm = wstr.tile([128, 8, 128], BF16, tag="w", name="wkm")
                            nc.sync.dma_start(
                                out=wkm,
                                in_=t["wk8"][m].rearrange("p (t n) -> p t n", t=8))
                            for qb in (qbs if isinstance(qbs, list) else [qbs]):
                                ps = kps.tile([128, 512], F32, tag="ps", name="kps")
                                for kt in range(8):
                                    nc.tensor.matmul(
                                        ps, wkm[:, kt, :],
                                        h1_fm[:, kt, qb * 512:(qb + 1) * 512],
                                        start=(kt == 0), stop=(kt == 7))
                                dst = ksb[:, m, qb * 512:(qb + 1) * 512]
                                if on_act:
                                    nc.scalar.activation(
                                        out=dst, in_=ps, func=AF.Identity,
                                        bias=bk_t[:, m:m + 1], scale=1.0)
                                else:
                                    nc.vector.tensor_scalar_add(
                                        out=dst, in0=ps,
                                        scalar1=bk_t[:, m:m + 1])

                    def q_pass(phases, on_act):
                        for m in range(8):
                            wqm = wstr.tile([128, 8, 128], BF16, tag="w", name="wqm")
                            nc.sync.dma_start(
                                out=wqm,
                                in_=t["wq8"][m].rearrange("p (t n) -> p t n", t=8))
                            for p in phases:
                                ps = qps.tile([128, 256], F32, tag="ps", name="qps")
                                for kt in range(8):
                                    nc.tensor.matmul(
                                        ps, wqm[:, kt, :],
                                        h1_fm[:, kt, 256 + 512 * p:512 + 512 * p],
                                        start=(kt == 0), stop=(kt == 7))
                                dst = q_fm[:, m, p * 256:(p + 1) * 256]
                                if on_act:
                                    nc.scalar.activation(
                                        out=dst, in_=ps, func=AF.Identity,
                                        bias=bq_t[:, m:m + 1], scale=1.0)
                                else:
                                    nc.vector.tensor_scalar_add(
                                        out=dst, in0=ps,
                                        scalar1=bq_t[:, m:m + 1])

                    for half in range(2):
                        t0 = 8 * half
                        on_act = (half == 0)
                        ln1_tile(t0, not on_act)
                        wvh0 = load_wv(0)
                        ln1_tile(t0 + 1, not on_act)
                        ln1_tile(t0 + 2, not on_act)
                        ln1_tile(t0 + 3, not on_act)
                        if half == 0:
                            nc.gpsimd.dma_start(out=bq_t, in_=t["bqd"])
                            nc.gpsimd.dma_start(out=bk_t, in_=t["bkd"])
                        for i in range(4):
                            ln1_tile(t0 + 4 + i, not on_act)
                            v_tile(wvh0, 0, t0 + i, on_act)
                        k_pass(2 * half, on_act)
                        for i in range(4, 8):
                            v_tile(wvh0, 0, t0 + i, on_act)
                        wvh1 = load_wv(1)
                        for i in range(8):
                            v_tile(wvh1, 1, t0 + i, on_act)
                        k_pass(2 * half + 1, on_act)
                        q_pass([2 * half, 2 * half + 1], on_act)
                    for p in range(4):
                        nc.sync.dma_start(
                            out=y_sb[:, 2 * p:2 * p + 2, :],
                            in_=bass.AP(tensor=t["xb"].tensor,
                                        offset=(256 + 512 * p) * D,
                                        ap=[[D, 128], [128 * D, 2], [1, D]]))

                # ===== P2: attention, 4 causal phases + fillers =====
                with tc.tile_pool(name="ctxp", bufs=1) as ctxp, \
                     tc.tile_pool(name="wop", bufs=1) as wop, \
                     tc.tile_pool(name="amsk", bufs=1) as mskp, \
                     tc.tile_pool(name="apt", bufs=4) as ptp, \
                     tc.tile_pool(name="anrm", bufs=6) as nrm, \
                     tc.tile_pool(name="aox", bufs=3) as oxp, \
                     tc.tile_pool(name="astps", bufs=2, space="PSUM") as stps, \
                     tc.tile_pool(name="acxps", bufs=1, space="PSUM") as cxps, \
                     tc.tile_pool(name="aops", bufs=1, space="PSUM") as opps:
                    ctx_fm = ctxp.tile([128, 8, OWN], BF16)
                    wo_t = wop.tile([128, 8, D], BF16)
                    bo_bc = wop.tile([128, D], F32, tag="bo", name="bo_bc")
                    mask_t = mskp.tile([128, 16, 256], BF16)
                    nc.gpsimd.dma_start(
                        out=mask_t,
                        in_=t["mskd"].rearrange("p (m q) -> p m q", m=16))
                    nc.gpsimd.dma_start(
                        out=wo_t, in_=t["wod"].rearrange("p (t n) -> p t n", t=8))
                    nc.gpsimd.dma_start(out=bo_bc, in_=_bcast(t["bo"]))

                    def ln2_tile(j):
                        h2_t = wrk2.tile([128, D], BF16, tag="h2t", name="h2t")
                        _ln_normalize(nc, stat2, eps_t, y_sb[:, j, :], h2_t, "b",
                                      on_dve=(j >= 2))
                        nc.sync.dma_start_transpose(
                            out=h2sb[j // 2][:, :, (j % 2) * 128:
                                             (j % 2) * 128 + 128], in_=h2_t)

                    def op_tile(j, n):
                        po = opps.tile([128, 512], F32, tag="po", name="po")
                        for kt in range(8):
                            nc.tensor.matmul(
                                po, ctx_fm[:, kt, j * 128:(j + 1) * 128],
                                wo_t[:, kt, n * 512:(n + 1) * 512],
                                start=(kt == 0), stop=(kt == 7))
                        yh = oxp.tile([128, 512], F32, tag="oy", name="yh")
                        nc.vector.tensor_add(
                            out=yh, in0=po, in1=bo_bc[:, n * 512:(n + 1) * 512])
                        nc.vector.tensor_add(
                            out=y_sb[:, j, n * 512:(n + 1) * 512],
                            in0=y_sb[:, j, n * 512:(n + 1) * 512], in1=yh)

                    pending = [None]  # deferred softmax finalize (pcx, p, h2)

                    def finalize_ctx():
                        if pending[0] is None:
                            return
                        pcx, fp, fh2 = pending[0]
                        pending[0] = None
                        for hh in range(2):
                            rec = nrm.tile([1, 256], F32, tag="rec", name="rec")
                            nc.vector.reciprocal(out=rec, in_=pcx[hh][64:65, :])
                            pb = nrm.tile([64, 256], F32, tag="pb", name="pb")
                            nc.gpsimd.partition_broadcast(pb, rec)
                            nc.vector.tensor_mul(
                                out=ctx_fm[hh * 64:(hh + 1) * 64, fh2,
                                           fp * 256:(fp + 1) * 256],
                                in0=pcx[hh][0:64, :], in1=pb)

                    def attn_phase(p, filler):
                        ext = EXT[p]
                        chunks = ext // 4
                        # masked chunk (diagonal, extra mask hop) first in the
                        # accumulation; short unmasked chain closes the group
                        order = [chunks - 1] + list(range(chunks - 1))
                        for h2 in range(8):
                            pcx = [cxps.tile([65, 256], F32, tag=f"cx{hh}",
                                             name=f"cx{hh}") for hh in range(2)]
                            pts = {}

                            def sc(hh, c):
                                pst = stps.tile([128, 4, 256], F32, tag="st",
                                                name="pst")
                                for j in range(4):
                                    kt = 4 * c + j
                                    nc.tensor.matmul(
                                        pst[:, j, :],
                                        ksb[hh * 64:(hh + 1) * 64, h2,
                                            kt * 128:(kt + 1) * 128],
                                        q_fm[hh * 64:(hh + 1) * 64, h2,
                                             p * 256:(p + 1) * 256])
                                pt = ptp.tile([128, 4, 256], BF16, tag="pt",
                                              name="pt")
                                nc.scalar.activation(out=pt, in_=pst, func=AF.Exp)
                                if c == chunks - 1:
                                    nc.vector.tensor_mul(
                                        out=pt, in0=pt,
                                        in1=mask_t[:, 4 * p:4 * p + 4, :])
                                pts[(hh, c)] = pt

                            def av(hh, ci):
                                h = 2 * h2 + hh
                                c = order[ci]
                                pt = pts.pop((hh, c))
                                for j in range(4):
                                    kt = 4 * c + j
                                    nc.tensor.matmul(
                                        pcx[hh], vaug[:, kt, h, :], pt[:, j, :],
                                        start=(ci == 0 and j == 0),
                                        stop=(ci == chunks - 1 and j == 3))

                            sc(0, order[0])
                            sc(1, order[0])
                            finalize_ctx()
                            filler(h2)
                            for ci in range(1, chunks):
                                av(0, ci - 1)
                                sc(0, order[ci])
                                av(1, ci - 1)
                                sc(1, order[ci])
                            av(0, chunks - 1)
                            av(1, chunks - 1)
                            pending[0] = (pcx, p, h2)

                    def mk_filler(g):
                        if g < 0:
                            return lambda h2: None
                        thunks = [
                            None,
                            lambda: op_tile(2 * g, 0),
                            lambda: op_tile(2 * g, 1),
                            lambda: ln2_tile(2 * g),
                            lambda: op_tile(2 * g + 1, 0),
                            lambda: op_tile(2 * g + 1, 1),
                            lambda: ln2_tile(2 * g + 1),
                            None,
                        ]

                        def filler(h2):
                            th = thunks[h2]
                            if th is not None:
                                th()
                        return filler

                    for p in range(4):
                        attn_phase(p, mk_filler(p - 1))
                    finalize_ctx()
                    # group 3 out-proj + LN2 (ctx/wo die with this scope)
                    for j in (6, 7):
                        op_tile(j, 0)
                        op_tile(j, 1)
                        ln2_tile(j)

            # ===== P4: MLP (single-pass w1 stream, resident w2) =====
            with tc.tile_pool(name="mw2", bufs=1) as w2p, \
                 tc.tile_pool(name="mw1", bufs=5) as w1str, \
                 tc.tile_pool(name="macts", bufs=1) as acts, \
                 tc.tile_pool(name="mout", bufs=3) as out4, \
                 tc.tile_pool(name="mps1", bufs=3, space="PSUM") as ps41, \
                 tc.tile_pool(name="mps2", bufs=3, space="PSUM") as ps42:
                w2_t = w2p.tile([128, 32, D], BF16)
                b2_bc = w2p.tile([128, D], F32, tag="b2", name="b2_bc")
                nc.gpsimd.dma_start(out=b1s_t, in_=t["b1d"])
                nc.gpsimd.dma_start(out=b2_bc, in_=_bcast(t["b2"]))
                a_c = [acts.tile([128, 32, 256], BF16, tag=f"a{c}", name=f"a{c}")
                       for c in range(4)]

                def fc1_pass(cs):
                    for f in range(32):
                        w1f = w1str.tile([128, 8, 128], BF16, tag="w1",
                                         name="w1f")
                        nc.gpsimd.dma_start(
                            out=w1f,
                            in_=t["w1t"][f].rearrange("p (t n) -> p t n", t=8))
                        for c in cs:
                            ps = ps41.tile([128, 256], F32, tag="ps", name="ps")
                            for kt in range(8):
                                nc.tensor.matmul(
                                    ps, w1f[:, kt, :],
                                    h2sb[c][:, kt, :],
                                    start=(kt == 0), stop=(kt == 7))
                            nc.scalar.activation(out=a_c[c][:, f, :], in_=ps,
                                                 func=AF.Silu, scale=1.702,
                                                 bias=b1s_t[:, f:f + 1])
                        if f % 6 == 0 and f > 0 and f <= 24 and cs[0] == 0:
                            q = f // 6 - 1
                            nc.sync.dma_start(
                                out=w2_t[:, 8 * q:8 * (q + 1), :],
                                in_=t["w2d"][:, q * 8192:(q + 1) * 8192]
                                .rearrange("p (a c) -> p a c", a=8))

                def fc2_chunk(c):
                    for t2 in range(2):
                        j = 2 * c + t2
                        for n in range(2):
                            py = ps42.tile([128, 512], F32, tag="py", name="py")
                            for kt in range(32):
                                nc.tensor.matmul(
                                    py, a_c[c][:, kt, t2 * 128:(t2 + 1) * 128],
                                    w2_t[:, kt, n * 512:(n + 1) * 512],
                                    start=(kt == 0), stop=(kt == 31))
                            ot = out4.tile([128, 512], F32, tag="ot", name="ot")
                            nc.vector.tensor_add(
                                out=ot, in0=py,
                                in1=b2_bc[:, n * 512:(n + 1) * 512])
                            nc.vector.tensor_add(
                                out=ot, in0=ot,
                                in1=y_sb[:, j, n * 512:(n + 1) * 512])
                            nc.scalar.dma_start(
                                out=t["yo"][j * 128:(j + 1) * 128,
                                            n * 512:(n + 1) * 512],
                                in_=ot)

                fc1_pass([0, 1, 2])
                fc2_chunk(0)
                fc1_pass([3])
                fc2_chunk(1)
                fc2_chunk(2)
                fc2_chunk(3)


def _perms():
    g = [np.arange(256 * i, 256 * (i + 1)) for i in range(8)]
    order = [[1, 0, 2, 3, 5, 4, 6, 7], [0, 1, 3, 2, 4, 5, 7, 6]]
    return [np.concatenate([g[i] for i in o]) for o in order]


def _masks(perm):
    """[128, 16*256] bf16: slot 4p+i covers key tile EXT[p]-4+i of phase p."""
    m = np.zeros((16, 128, 256), np.float32)
    for p in range(4):
        qg = perm[256 + 512 * p:512 + 512 * p]
        for i in range(4):
            kt = EXT[p] - 4 + i
            kg = perm[kt * 128:(kt + 1) * 128]
            m[4 * p + i] = (kg[:, None] <= qg[None, :]).astype(np.float32)
    return np.ascontiguousarray(
        m.transpose(1, 0, 2).reshape(128, 16 * 256)).astype(ml_dtypes.bfloat16)


def _perm_w_mtiles(W, mt):
    """[Din, Dout] -> [mt, 128, Din//128 * (Dout//mt)]."""
    din, dout = W.shape
    n_sz = dout // mt
    A = W.reshape(din // 128, 128, mt, n_sz)
    return np.ascontiguousarray(A.transpose(2, 1, 0, 3).reshape(mt, 128, -1))


def _prep_consts(inputs):
    f = {k: np.asarray(v, np.float64) for k, v in inputs.items()}
    g1, b1 = f["ln1_g"], f["ln1_b"]
    g2, b2 = f["ln2_g"], f["ln2_b"]
    qs = 1.0 / np.sqrt(HD)
    wq = ((g1[:, None] * f["Wq"]) * qs).astype(np.float32)
    wk = (g1[:, None] * f["Wk"]).astype(np.float32)
    wv = (g1[:, None] * f["Wv"]).astype(np.float32)
    w1 = (g2[:, None] * f["W1"]).astype(np.float32)
    bf = ml_dtypes.bfloat16
    c = {}
    c["wq8"] = _perm_w_mtiles(wq, 8).astype(bf)
    c["wk8"] = _perm_w_mtiles(wk, 8).astype(bf)
    c["wvd"] = np.ascontiguousarray(
        wv.reshape(8, 128, 2, 512).transpose(1, 2, 0, 3)).astype(bf)
    c["wod"] = np.ascontiguousarray(
        f["Wo"].astype(np.float32).reshape(8, 128, D).transpose(1, 0, 2)
        .reshape(128, 8 * D)).astype(bf)
    c["w1t"] = _perm_w_mtiles(w1, 32).astype(bf)
    c["w2d"] = np.ascontiguousarray(
        (f["W2"] / 1.702).astype(np.float32)
        .reshape(32, 128, D).transpose(1, 0, 2).reshape(128, 32 * D)).astype(bf)
    c["bqd"] = np.ascontiguousarray(
        ((b1 @ f["Wq"] + f["bq"]) * qs).astype(np.float32).reshape(8, 128).T)
    c["bkd"] = np.ascontiguousarray(
        (b1 @ f["Wk"] + f["bk"]).astype(np.float32).reshape(8, 128).T)
    c["b1d"] = np.ascontiguousarray(
        (1.702 * (b2 @ f["W1"] + f["b1"])).astype(np.float32).reshape(32, 128).T)
    bv_eff = b1 @ f["Wv"] + f["bv"]
    c["bo"] = (bv_eff @ f["Wo"] + f["bo"]).astype(np.float32)
    c["b2"] = f["b2"].astype(np.float32)
    return c


def kernel(**inputs):
    if "nc" not in _CACHE:
        _CACHE["nc"] = _build_program()
        _CACHE["perms"] = _perms()
        _CACHE["masks"] = [_masks(p) for p in _CACHE["perms"]]
    nc = _CACHE["nc"]
    perms, masks = _CACHE["perms"], _CACHE["masks"]

    x = np.asarray(inputs["x"], np.float32)
    c = _prep_consts(inputs)

    in_maps = []
    for core in range(NCORES):
        b, s = core // 2, core % 2
        m = dict(c)
        m["xb"] = np.ascontiguousarray(x[b][perms[s]])
        m["xh"] = m["xb"].astype(ml_dtypes.bfloat16)
        m["mskd"] = masks[s]
        in_maps.append(m)

    res = run_bass_kernel_spmd(nc, in_maps, core_ids=list(range(NCORES)))

    own_local = np.concatenate(
        [np.arange(256 + 512 * p, 512 + 512 * p) for p in range(4)])
    out = np.empty((B, S, D), np.float32)
    for core in range(NCORES):
        b, s = core // 2, core % 2
        out[b][perms[s][own_local]] = res.results[core]["yo"]
    return out
